# revision 12
# baseline (speedup 1.0000x reference)
"""Causal self-attention on 8 NeuronCores.

Sharding: data-parallel over batch (B=2) x tensor-parallel over heads
(16 heads -> 4 groups of 4), Megatron-style. Core c handles batch c//4,
head-group c%4. Each core computes its QKV projection slice, 4 heads of
causal attention, and a partial output projection; the host sums the 4
partials per batch element.

Per-core dataflow (S=2048, D=1024, HD=64, 4 local heads):
  x [S,D] --PE transpose--> xT [D,S]
  qkT = W_qk^T x^T  (fp32r matmuls, heads packed 2-per-128-partitions)
  v   = x W_v       (natural layout, + ones column for row-sum trick)
  scoresT[sk,sq] = kT^T qT ; exp on ACT ; causal mask via affine_select
  yT|sums = [v|1]^T @ probsT  (PV matmul fuses softmax denominator)
  y = yT / sums ; partial_out = y^T W_proj + b_proj (b_proj on group-0 cores)
"""

import sys

sys.path.insert(0, "/opt/trn_rl_repo")

import numpy as np

import concourse.bass as bass
import concourse.bacc as bacc
import concourse.mybir as mybir
from concourse.bass import ts, ds
from concourse.masks import make_identity
from concourse.tile import TileContext

B, S, D, H = 2, 2048, 1024, 16
HD = D // H  # 64
NH = 4  # heads per core
P = 128
DT = D // P  # 8 d-tiles
ST = S // P  # 16 s-tiles
NCHUNK = 4  # sq chunks of 512
CH = 512
F32 = mybir.dt.float32
F32R = mybir.dt.float32r
LOOKAHEAD = 3

_nc_cache = None


def r(ap):
    return ap.bitcast(F32R)


def build_nc():
    nc = bacc.Bacc("TRN2", target_bir_lowering=False, debug=False, num_devices=8)

    x_d = nc.dram_tensor("x", [S, D], F32, kind="ExternalInput")
    wqk_d = nc.dram_tensor("wqk", [D, 2 * NH * HD], F32R, kind="ExternalInput")
    wv_d = nc.dram_tensor("wv", [D, NH * HD], F32R, kind="ExternalInput")
    wp_d = nc.dram_tensor("wp", [NH * HD, D], F32R, kind="ExternalInput")
    bqk_d = nc.dram_tensor("bqk", [2 * NH * HD], F32, kind="ExternalInput")
    bv_d = nc.dram_tensor("bv", [NH * (HD + 1)], F32, kind="ExternalInput")
    bp_d = nc.dram_tensor("bp", [D], F32, kind="ExternalInput")
    out_d = nc.dram_tensor("out", [S, D], F32, kind="ExternalOutput")

    with TileContext(nc) as tc:
        import contextlib

        stack = contextlib.ExitStack()
        with stack:
            consts = stack.enter_context(tc.tile_pool(name="consts", bufs=1))
            bigs = stack.enter_context(tc.tile_pool(name="bigs", bufs=1))

            # ---- constants ----
            wp_sb = consts.tile([P, 2, D], F32R)
            nc.sync.dma_start(wp_sb, wp_d[:].rearrange("(i p) f -> p i f", p=P))
            bqk_sb = consts.tile([P, 4], F32)
            nc.sync.dma_start(bqk_sb, bqk_d[:].rearrange("(t p) -> p t", p=P))
            bv_sb = consts.tile([1, NH * (HD + 1)], F32)
            nc.sync.dma_start(bv_sb, bv_d[:].unsqueeze(0))
            bv_rep = consts.tile([P, NH * (HD + 1)], F32)
            nc.gpsimd.partition_broadcast(bv_rep, bv_sb)
            bp_sb = consts.tile([1, D], F32)
            nc.sync.dma_start(bp_sb, bp_d[:].unsqueeze(0))
            bp_rep = consts.tile([P, D], F32)
            nc.gpsimd.partition_broadcast(bp_rep, bp_sb)
            ident = consts.tile([P, P], F32)
            make_identity(nc, ident)

            # ---- persistent activations (live across both stages) ----
            qkT_sb = bigs.tile([P, 4, S], F32R)  # q: tiles 0-1, k: tiles 2-3
            v_sb = bigs.tile([P, ST, NH * (HD + 1)], F32R)  # [s%128, s//128, h*65+c]

            v4 = v_sb.rearrange("p st (h c) -> p st h c", c=HD + 1)
            # ones column for the fused row-sum: out = src*0 + 1 (memset can't
            # write f32r; DVE rounds on write)
            nc.vector.tensor_scalar(
                out=v4[:, :, :, HD : HD + 1],
                in0=bv_rep[:, 0:1, None, None].to_broadcast((P, ST, NH, 1)),
                scalar1=0.0,
                scalar2=1.0,
                op0=mybir.AluOpType.mult,
                op1=mybir.AluOpType.add,
            )

            # ================= stage A: transpose x + projections ==========
            with (
                tc.tile_pool(name="stageA", bufs=1) as pa,
                tc.tile_pool(name="psA", bufs=1, space="PSUM") as psA,
            ):
                wqk_sb = pa.tile([P, DT, 2 * NH * HD], F32R)
                nc.sync.dma_start(
                    wqk_sb, wqk_d[:].rearrange("(do p) f -> p do f", p=P)
                )
                wv_sb = pa.tile([P, DT, NH * HD], F32R)
                nc.sync.dma_start(wv_sb, wv_d[:].rearrange("(do p) f -> p do f", p=P))
                xT_sb = pa.tile([P, DT, S], F32R)  # [d%128, d//128, s]
                for sg in range(NCHUNK):
                    for st4 in range(4):
                        st = 4 * sg + st4
                        x_t = pa.tile([P, D], F32, name="x_t", bufs=2)
                        nc.sync.dma_start(x_t, x_d[ts(st, P), :])
                        for bank in range(2):
                            tp = psA.tile([P, CH], F32, name="tp", tag="tp", bufs=2)
                            for q in range(4):
                                do = 4 * bank + q
                                nc.tensor.transpose(
                                    tp[:, ts(q, P)], x_t[:, ts(do, P)], ident
                                )
                            nc.vector.tensor_copy(
                                out=xT_sb[:, 4 * bank : 4 * bank + 4, ts(st, P)],
                                in_=tp.rearrange("p (a b) -> p a b", b=P),
                            )
                    # qk projection for this 512-wide s-chunk
                    for ft in range(4):
                        pp = psA.tile([P, CH], F32, name="pp", tag="proj", bufs=2)
                        for do in range(DT):
                            nc.tensor.matmul(
                                pp,
                                (wqk_sb[:, do, ts(ft, P)]),
                                (xT_sb[:, do, ds(sg * CH, CH)]),
                                start=(do == 0),
                                stop=(do == DT - 1),
                            )
                        nc.vector.tensor_scalar_add(
                            qkT_sb[:, ft, ds(sg * CH, CH)], pp, bqk_sb[:, ft : ft + 1]
                        )
                    # v projection for the 4 s-tiles of this chunk
                    for st4 in range(4):
                        st = 4 * sg + st4
                        pv = psA.tile([P, NH * HD], F32, name="pv", tag="projv", bufs=2)
                        for do in range(DT):
                            nc.tensor.matmul(
                                pv,
                                (xT_sb[:, do, ts(st, P)]),
                                (wv_sb[:, do, :]),
                                start=(do == 0),
                                stop=(do == DT - 1),
                            )
                        nc.vector.tensor_tensor(
                            v4[:, st, :, :HD],
                            pv.rearrange("p (h c) -> p h c", c=HD),
                            bv_rep.rearrange("p (h c) -> p h c", c=HD + 1)[:, :, :HD],
                            mybir.AluOpType.add,
                        )

            # ================= stage B: attention + output projection ======
            with (
                tc.tile_pool(name="stageB", bufs=1) as pb,
                tc.tile_pool(name="psB", bufs=1, space="PSUM") as psB,
            ):
                yT_sb = pb.tile([P, 2, S], F32R)  # [dh%128, dh//128, s]
                for j in range(NCHUNK):
                    nsk = 4 * j + 4
                    # band tiles (need masking) first, then the dense tiles
                    seq = list(range(4 * j, nsk)) + list(range(0, 4 * j))
                    for h in range(NH):
                        po = (h % 2) * 64
                        qa = qkT_sb[po : po + 64, h // 2, ds(j * CH, CH)]
                        pvp = psB.tile([P, CH], F32, name="pvp", tag="pv", bufs=2)
                        pts = {}

                        def qk_one(idx):
                            i = seq[idx]
                            sc = psB.tile([P, CH], F32, name="sc", tag="sc", bufs=4)
                            nc.tensor.matmul(
                                sc,
                                (qkT_sb[po : po + 64, 2 + h // 2, ts(i, P)]),
                                (qa),
                                start=True,
                                stop=True,
                            )
                            pt = pb.tile([P, CH], F32R, name="pt", tag="pt", bufs=5)
                            nc.scalar.activation(
                                pt, sc, mybir.ActivationFunctionType.Exp, scale=0.125
                            )
                            if i >= 4 * j:
                                p_rel = i - 4 * j
                                # keep where (col - row - 128*p_rel) >= 0
                                nc.gpsimd.affine_select(
                                    out=pt,
                                    in_=pt,
                                    compare_op=mybir.AluOpType.is_ge,
                                    fill=0.0,
                                    base=-P * p_rel,
                                    pattern=[[1, CH]],
                                    channel_multiplier=-1,
                                )
                            pts[idx] = pt

                        for idx in range(min(LOOKAHEAD, nsk)):
                            qk_one(idx)
                        for idx in range(nsk):
                            if idx + LOOKAHEAD < nsk:
                                qk_one(idx + LOOKAHEAD)
                            i = seq[idx]
                            nc.tensor.matmul(
                                pvp[: HD + 1],
                                (v_sb[:, i, ds(h * (HD + 1), HD + 1)]),
                                (pts.pop(idx)),
                                start=(idx == 0),
                                stop=(idx == nsk - 1),
                            )
                        rec = pb.tile([1, CH], F32, name="rec", bufs=2)
                        nc.vector.reciprocal(rec, pvp[HD : HD + 1, :])
                        rec_rep = pb.tile([HD, CH], F32, name="rec_rep", bufs=2)
                        nc.gpsimd.partition_broadcast(rec_rep, rec)
                        nc.vector.tensor_tensor(
                            yT_sb[po : po + 64, h // 2, ds(j * CH, CH)],
                            pvp[0:HD, :],
                            rec_rep,
                            mybir.AluOpType.mult,
                        )
                    # output projection for the 4 s-tiles of this chunk
                    for st4 in range(4):
                        st = 4 * j + st4
                        o_t = pb.tile([P, D], F32, name="o_t", bufs=2)
                        for c in range(2):
                            op = psB.tile([P, CH], F32, name="op", tag="out", bufs=2)
                            for i2 in range(2):
                                nc.tensor.matmul(
                                    op,
                                    (yT_sb[:, i2, ts(st, P)]),
                                    (wp_sb[:, i2, ds(c * CH, CH)]),
                                    start=(i2 == 0),
                                    stop=(i2 == 1),
                                )
                            nc.vector.tensor_tensor(
                                o_t[:, ds(c * CH, CH)],
                                op,
                                bp_rep[:, ds(c * CH, CH)],
                                mybir.AluOpType.add,
                            )
                        nc.sync.dma_start(out_d[ts(st, P), :], o_t)

    nc.compile()
    return nc


def make_in_maps(x, W_attn, b_attn, W_proj, b_proj):
    x = np.ascontiguousarray(np.asarray(x, dtype=np.float32))
    W_attn = np.asarray(W_attn, dtype=np.float32)
    b_attn = np.asarray(b_attn, dtype=np.float32)
    W_proj = np.asarray(W_proj, dtype=np.float32)
    b_proj = np.asarray(b_proj, dtype=np.float32)
    GF = NH * HD  # 256 features per group
    in_maps = []
    for c in range(8):
        b, g = divmod(c, 4)
        sl = slice(g * GF, (g + 1) * GF)
        wqk = np.concatenate(
            [W_attn[:, sl], W_attn[:, D + g * GF : D + (g + 1) * GF]], axis=1
        )
        bqk = np.concatenate([b_attn[sl], b_attn[D + g * GF : D + (g + 1) * GF]])
        wv = W_attn[:, 2 * D + g * GF : 2 * D + (g + 1) * GF]
        bv_flat = b_attn[2 * D + g * GF : 2 * D + (g + 1) * GF]
        bv = np.zeros(NH * (HD + 1), dtype=np.float32)
        for h in range(NH):
            bv[h * (HD + 1) : h * (HD + 1) + HD] = bv_flat[h * HD : (h + 1) * HD]
        in_maps.append(
            {
                "x": np.ascontiguousarray(x[b]),
                "wqk": np.ascontiguousarray(wqk),
                "wv": np.ascontiguousarray(wv),
                "wp": np.ascontiguousarray(W_proj[sl, :]),
                "bqk": np.ascontiguousarray(bqk),
                "bv": bv,
                "bp": (b_proj if g == 0 else np.zeros_like(b_proj)).copy(),
            }
        )
    return in_maps


def kernel(x, W_attn, b_attn, W_proj, b_proj):
    global _nc_cache
    from concourse.bass_utils import run_bass_kernel_spmd

    if _nc_cache is None:
        _nc_cache = build_nc()
    nc = _nc_cache
    in_maps = make_in_maps(x, W_attn, b_attn, W_proj, b_proj)
    res = run_bass_kernel_spmd(nc, in_maps, core_ids=list(range(8)))
    out = np.zeros((B, S, D), dtype=np.float32)
    for c in range(8):
        b = c // 4
        out[b] += res.results[c]["out"]
    return out


# revision 20
# speedup vs baseline: 2.1770x; 2.1770x over previous
"""Causal self-attention on 8 NeuronCores.

Sharding: data-parallel over batch (B=2) x tensor-parallel over heads
(16 heads -> 4 groups of 4), Megatron-style. Core c handles batch c//4,
head-group c%4. Each core computes its QKV projection slice, 4 heads of
causal attention, and a partial output projection; the host sums the 4
partials per batch element.

Per-core dataflow (S=2048, D=1024, HD=64, 4 local heads):
  x [S,D] --PE transpose--> xT [D,S]
  qkT = W_qk^T x^T  (fp32r matmuls, heads packed 2-per-128-partitions)
  v   = x W_v       (natural layout, + ones column for row-sum trick)
  scoresT[sk,sq] = kT^T qT ; exp on ACT ; causal mask via affine_select
  yT|sums = [v|1]^T @ probsT  (PV matmul fuses softmax denominator)
  y = yT / sums ; partial_out = y^T W_proj + b_proj (b_proj on group-0 cores)
"""

import sys

sys.path.insert(0, "/opt/trn_rl_repo")

import numpy as np

import concourse.bass as bass
import concourse.bacc as bacc
import concourse.mybir as mybir
from concourse.bass import ts, ds
from concourse.masks import make_identity
from concourse.tile import TileContext

B, S, D, H = 2, 2048, 1024, 16
HD = D // H  # 64
NH = 4  # heads per core
P = 128
DT = D // P  # 8 d-tiles
ST = S // P  # 16 s-tiles
NCHUNK = 4  # sq chunks of 512
CH = 512
F32 = mybir.dt.float32
F32R = mybir.dt.float32r
LOOKAHEAD = 3

_nc_cache = None


def r(ap):
    return ap.bitcast(F32R)


def build_nc():
    nc = bacc.Bacc("TRN2", target_bir_lowering=False, debug=False, num_devices=8)

    x_d = nc.dram_tensor("x", [S, D], F32, kind="ExternalInput")
    wqk_d = nc.dram_tensor("wqk", [D, 2 * NH * HD], F32R, kind="ExternalInput")
    wv_d = nc.dram_tensor("wv", [D, NH * HD], F32R, kind="ExternalInput")
    wp_d = nc.dram_tensor("wp", [NH * HD, D], F32R, kind="ExternalInput")
    bqk_d = nc.dram_tensor("bqk", [2 * NH * HD], F32, kind="ExternalInput")
    bv_d = nc.dram_tensor("bv", [NH * (HD + 1)], F32, kind="ExternalInput")
    bp_d = nc.dram_tensor("bp", [D], F32, kind="ExternalInput")
    out_d = nc.dram_tensor("out", [S, D], F32, kind="ExternalOutput")

    with TileContext(nc) as tc:
        import contextlib

        stack = contextlib.ExitStack()
        with stack:
            consts = stack.enter_context(tc.tile_pool(name="consts", bufs=1))
            bigs = stack.enter_context(tc.tile_pool(name="bigs", bufs=1))

            # ---- constants (tiles; DMAs are placed to keep x loads first) ----
            wp_sb = consts.tile([P, 2, D], F32R)
            bqk_sb = consts.tile([P, 4], F32)
            bv_sb = consts.tile([1, NH * (HD + 1)], F32)
            bv_rep = consts.tile([P, NH * (HD + 1)], F32)
            bp_sb = consts.tile([1, D], F32)
            bp_rep = consts.tile([P, D], F32)
            ident = consts.tile([P, P], F32)
            make_identity(nc, ident)
            ones_col = consts.tile([1, HD], F32R)
            nc.vector.tensor_scalar(
                out=ones_col,
                in0=ident[0:1, :HD],
                scalar1=0.0,
                scalar2=1.0,
                op0=mybir.AluOpType.mult,
                op1=mybir.AluOpType.add,
            )

            # ---- persistent activations (live across both stages) ----
            qkT_sb = bigs.tile([P, 4, S], F32R)  # q: tiles 0-1, k: tiles 2-3
            v_sb = bigs.tile([P, ST, NH * (HD + 1)], F32R)  # [s%128, s//128, h*65+c]

            v4 = v_sb.rearrange("p st (h c) -> p st h c", c=HD + 1)
            # ones column for the fused row-sum: out = src*0 + 1 (memset can't
            # write f32r; DVE rounds on write). ident is just an initialized src.
            nc.vector.tensor_scalar(
                out=v4[:, :, :, HD : HD + 1],
                in0=ident[:, 0:1, None, None].to_broadcast((P, ST, NH, 1)),
                scalar1=0.0,
                scalar2=1.0,
                op0=mybir.AluOpType.mult,
                op1=mybir.AluOpType.add,
            )

            # ================= stage A: transpose x + projections ==========
            with (
                tc.tile_pool(name="stageA", bufs=1) as pa,
                tc.tile_pool(name="psA", bufs=1, space="PSUM") as psA,
            ):
                wqk_sb = pa.tile([P, DT, 2 * NH * HD], F32R)
                wv_sb = pa.tile([P, DT, NH * HD], F32R)
                xT_sb = pa.tile([P, DT, S], F32R)  # [d%128, d//128, s]
                for sg in range(NCHUNK):
                    for st4 in range(4):
                        st = 4 * sg + st4
                        x_t = pa.tile([P, D], F32, name="x_t", bufs=4)
                        nc.sync.dma_start(x_t[:, :CH], x_d[ts(st, P), :CH])
                        nc.sync.dma_start(x_t[:, CH:], x_d[ts(st, P), CH:])
                        if sg == 0 and st4 == 1:
                            nc.sync.dma_start(
                                wqk_sb, wqk_d[:].rearrange("(do p) f -> p do f", p=P)
                            )
                        if sg == 0 and st4 == 2:
                            nc.sync.dma_start(
                                wv_sb, wv_d[:].rearrange("(do p) f -> p do f", p=P)
                            )
                        if sg == 0 and st4 == 3:
                            nc.sync.dma_start(
                                bqk_sb, bqk_d[:].rearrange("(t p) -> p t", p=P)
                            )
                            nc.sync.dma_start(bv_sb, bv_d[:].unsqueeze(0))
                            nc.gpsimd.partition_broadcast(bv_rep, bv_sb)
                        for bank in range(2):
                            tp = psA.tile([P, CH], F32, name="tp", tag="tp", bufs=2)
                            for q in range(4):
                                do = 4 * bank + q
                                nc.tensor.transpose(
                                    tp[:, ts(q, P)], x_t[:, ts(do, P)], ident
                                )
                            nc.vector.tensor_copy(
                                out=xT_sb[:, 4 * bank : 4 * bank + 4, ts(st, P)],
                                in_=tp.rearrange("p (a b) -> p a b", b=P),
                            )
                    # qk projection for this 512-wide s-chunk
                    for ft in range(4):
                        pp = psA.tile([P, CH], F32, name="pp", tag="proj", bufs=2)
                        for do in range(DT):
                            nc.tensor.matmul(
                                pp,
                                (wqk_sb[:, do, ts(ft, P)]),
                                (xT_sb[:, do, ds(sg * CH, CH)]),
                                start=(do == 0),
                                stop=(do == DT - 1),
                            )
                        nc.vector.tensor_scalar_add(
                            qkT_sb[:, ft, ds(sg * CH, CH)], pp, bqk_sb[:, ft : ft + 1]
                        )
                    # v projection for the 4 s-tiles of this chunk
                    for st4 in range(4):
                        st = 4 * sg + st4
                        pv = psA.tile([P, NH * HD], F32, name="pv", tag="projv", bufs=2)
                        for do in range(DT):
                            nc.tensor.matmul(
                                pv,
                                (xT_sb[:, do, ts(st, P)]),
                                (wv_sb[:, do, :]),
                                start=(do == 0),
                                stop=(do == DT - 1),
                            )
                        nc.vector.tensor_tensor(
                            v4[:, st, :, :HD],
                            pv.rearrange("p (h c) -> p h c", c=HD),
                            bv_rep.rearrange("p (h c) -> p h c", c=HD + 1)[:, :, :HD],
                            mybir.AluOpType.add,
                        )

            # ================= stage B: attention + output projection ======
            with (
                tc.tile_pool(name="stageB", bufs=1) as pb,
                tc.tile_pool(name="psB", bufs=1, space="PSUM") as psB,
            ):
                nc.sync.dma_start(wp_sb, wp_d[:].rearrange("(i p) f -> p i f", p=P))
                nc.sync.dma_start(bp_sb, bp_d[:].unsqueeze(0))
                nc.gpsimd.partition_broadcast(bp_rep, bp_sb)
                yT_sb = pb.tile([P, 2, S], F32R)  # [dh%128, dh//128, s]
                LP = 2  # pair lookahead
                for j in range(NCHUNK):
                    nsk = 4 * j + 4
                    npair = nsk // 2
                    # band pairs (need masking) first, then the dense pairs
                    pairs = [(4 * j, 4 * j + 1), (4 * j + 2, 4 * j + 3)] + [
                        (i, i + 1) for i in range(0, 4 * j, 2)
                    ]
                    o_ts = {}

                    def emit_outproj(i2):
                        # half-projection over dh tile i2 (heads 2*i2, 2*i2+1)
                        for st4 in range(4):
                            st = 4 * j + st4
                            if i2 == 0:
                                o_ts[st4] = pb.tile([P, D], F32, name="o_t", bufs=5)
                            o_t = o_ts[st4]
                            for c in range(2):
                                op = psB.tile(
                                    [P, CH], F32, name="op", tag="pv", bufs=2
                                )
                                nc.tensor.matmul(
                                    op,
                                    (yT_sb[:, i2, ts(st, P)]),
                                    (wp_sb[:, i2, ds(c * CH, CH)]),
                                    start=True,
                                    stop=True,
                                )
                                if i2 == 0:
                                    nc.vector.tensor_tensor(
                                        o_t[:, ds(c * CH, CH)],
                                        op,
                                        bp_rep[:, ds(c * CH, CH)],
                                        mybir.AluOpType.add,
                                    )
                                else:
                                    nc.vector.tensor_tensor(
                                        o_t[:, ds(c * CH, CH)],
                                        o_t[:, ds(c * CH, CH)],
                                        op,
                                        mybir.AluOpType.add,
                                    )
                            if i2 == 1:
                                nc.sync.dma_start(out_d[ts(st, P), :], o_t)

                    pending_norm = []

                    def flush_norm():
                        # bc matmul + normalize for heads whose recip is ready;
                        # deferred so the PE never waits on the DVE chain
                        while pending_norm:
                            hN, ysbN, recN = pending_norm.pop(0)
                            bc = psB.tile([P, CH], F32, name="bc", tag="pv", bufs=2)
                            nc.tensor.matmul(
                                bc[:HD], ones_col, recN, start=True, stop=True
                            )
                            nc.vector.tensor_tensor(
                                yT_sb[
                                    (hN % 2) * 64 : (hN % 2) * 64 + 64,
                                    hN // 2,
                                    ds(j * CH, CH),
                                ],
                                ysbN[0:HD, :],
                                bc[:HD],
                                mybir.AluOpType.mult,
                            )

                    for h in range(NH):
                        po = (h % 2) * 64
                        qa = qkT_sb[po : po + 64, h // 2, ds(j * CH, CH)]
                        pvp = psB.tile([P, CH], F32, name="pvp", tag="pv", bufs=2)
                        pts = {}

                        def qk_pair(pidx):
                            i0, i1 = pairs[pidx]
                            sc = psB.tile(
                                [P, 2 * CH], F32, name="sc", tag="sc", bufs=3
                            )
                            for half, i in enumerate((i0, i1)):
                                nc.tensor.matmul(
                                    sc[:, ds(half * CH, CH)],
                                    (qkT_sb[po : po + 64, 2 + h // 2, ts(i, P)]),
                                    (qa),
                                    start=True,
                                    stop=True,
                                )
                            pt = pb.tile(
                                [P, 2 * CH], F32R, name="pt", tag="pt", bufs=3
                            )
                            nc.scalar.activation(
                                pt, sc, mybir.ActivationFunctionType.Exp, scale=0.125
                            )
                            for half, i in enumerate((i0, i1)):
                                if i >= 4 * j:
                                    p_rel = i - 4 * j
                                    # keep where (col - row - 128*p_rel) >= 0
                                    nc.gpsimd.affine_select(
                                        out=pt[:, ds(half * CH, CH)],
                                        in_=pt[:, ds(half * CH, CH)],
                                        compare_op=mybir.AluOpType.is_ge,
                                        fill=0.0,
                                        base=-P * p_rel,
                                        pattern=[[1, CH]],
                                        channel_multiplier=-1,
                                    )
                            pts[pidx] = pt

                        for pidx in range(min(LP, npair)):
                            qk_pair(pidx)
                        if pending_norm:
                            flush_norm()
                        if h == 2:
                            emit_outproj(0)
                        for pidx in range(npair):
                            if pidx + LP < npair:
                                qk_pair(pidx + LP)
                            i0, i1 = pairs[pidx]
                            pt = pts.pop(pidx)
                            nc.tensor.matmul(
                                pvp[: HD + 1],
                                (v_sb[:, i0, ds(h * (HD + 1), HD + 1)]),
                                pt[:, :CH],
                                start=(pidx == 0),
                                stop=False,
                            )
                            nc.tensor.matmul(
                                pvp[: HD + 1],
                                (v_sb[:, i1, ds(h * (HD + 1), HD + 1)]),
                                pt[:, CH:],
                                start=False,
                                stop=(pidx == npair - 1),
                            )
                        ysb = pb.tile([HD + 1, CH], F32, name="ysb", bufs=3)
                        nc.vector.tensor_copy(ysb, pvp[: HD + 1, :])
                        rec = pb.tile([1, CH], F32R, name="rec", bufs=2)
                        with nc.allow_low_precision(reason="recip feeds f32r bcast"):
                            nc.vector.reciprocal(rec, ysb[HD : HD + 1, :])
                        pending_norm.append((h, ysb, rec))
                    flush_norm()
                    emit_outproj(1)

    nc.compile()
    return nc


def make_in_maps(x, W_attn, b_attn, W_proj, b_proj):
    x = np.ascontiguousarray(np.asarray(x, dtype=np.float32))
    W_attn = np.asarray(W_attn, dtype=np.float32)
    b_attn = np.asarray(b_attn, dtype=np.float32)
    W_proj = np.asarray(W_proj, dtype=np.float32)
    b_proj = np.asarray(b_proj, dtype=np.float32)
    GF = NH * HD  # 256 features per group
    in_maps = []
    for c in range(8):
        b, g = divmod(c, 4)
        sl = slice(g * GF, (g + 1) * GF)
        wqk = np.concatenate(
            [W_attn[:, sl], W_attn[:, D + g * GF : D + (g + 1) * GF]], axis=1
        )
        bqk = np.concatenate([b_attn[sl], b_attn[D + g * GF : D + (g + 1) * GF]])
        wv = W_attn[:, 2 * D + g * GF : 2 * D + (g + 1) * GF]
        bv_flat = b_attn[2 * D + g * GF : 2 * D + (g + 1) * GF]
        bv = np.zeros(NH * (HD + 1), dtype=np.float32)
        for h in range(NH):
            bv[h * (HD + 1) : h * (HD + 1) + HD] = bv_flat[h * HD : (h + 1) * HD]
        in_maps.append(
            {
                "x": np.ascontiguousarray(x[b]),
                "wqk": np.ascontiguousarray(wqk),
                "wv": np.ascontiguousarray(wv),
                "wp": np.ascontiguousarray(W_proj[sl, :]),
                "bqk": np.ascontiguousarray(bqk),
                "bv": bv,
                "bp": (b_proj if g == 0 else np.zeros_like(b_proj)).copy(),
            }
        )
    return in_maps


def kernel(x, W_attn, b_attn, W_proj, b_proj):
    global _nc_cache
    from concourse.bass_utils import run_bass_kernel_spmd

    if _nc_cache is None:
        _nc_cache = build_nc()
    nc = _nc_cache
    in_maps = make_in_maps(x, W_attn, b_attn, W_proj, b_proj)
    res = run_bass_kernel_spmd(nc, in_maps, core_ids=list(range(8)))
    out = np.zeros((B, S, D), dtype=np.float32)
    for c in range(8):
        b = c // 4
        out[b] += res.results[c]["out"]
    return out


# revision 22
# speedup vs baseline: 613.8150x; 281.9487x over previous
"""Causal self-attention on 8 NeuronCores.

Sharding: data-parallel over batch (B=2) x tensor-parallel over heads
(16 heads -> 4 groups of 4), Megatron-style. Core c handles batch c//4,
head-group c%4. Each core computes its QKV projection slice, 4 heads of
causal attention, and a partial output projection; the host sums the 4
partials per batch element.

Per-core dataflow (S=2048, D=1024, HD=64, 4 local heads):
  x [S,D] --PE transpose--> xT [D,S]
  qkT = W_qk^T x^T  (fp32r matmuls, heads packed 2-per-128-partitions)
  v   = x W_v       (natural layout, + ones column for row-sum trick)
  scoresT[sk,sq] = kT^T qT ; exp on ACT ; causal mask via affine_select
  yT|sums = [v|1]^T @ probsT  (PV matmul fuses softmax denominator)
  y = yT / sums ; partial_out = y^T W_proj + b_proj (b_proj on group-0 cores)
"""

import sys

sys.path.insert(0, "/opt/trn_rl_repo")

import numpy as np

import concourse.bass as bass
import concourse.bacc as bacc
import concourse.mybir as mybir
from concourse.bass import ts, ds
from concourse.masks import make_identity
from concourse.tile import TileContext

B, S, D, H = 2, 2048, 1024, 16
HD = D // H  # 64
NH = 4  # heads per core
P = 128
DT = D // P  # 8 d-tiles
ST = S // P  # 16 s-tiles
NCHUNK = 4  # sq chunks of 512
CH = 512
F32 = mybir.dt.float32
F32R = mybir.dt.float32r
XT_BUFS = 4
SC_BUFS = 3
PT_BUFS = 14
PV_BUFS = 2
TP_BUFS = 3
PROJ_BUFS = 2
PROJV_BUFS = 2
YSB_BUFS = 3
OT_BUFS = 5
REC_BUFS = 2
LP_PAIRS = 2

_nc_cache = None


def r(ap):
    return ap.bitcast(F32R)


def build_nc():
    nc = bacc.Bacc("TRN2", target_bir_lowering=False, debug=False, num_devices=8)

    x_d = nc.dram_tensor("x", [S, D], F32, kind="ExternalInput")
    wqk_d = nc.dram_tensor("wqk", [D, 2 * NH * HD], F32R, kind="ExternalInput")
    wv_d = nc.dram_tensor("wv", [D, NH * HD], F32R, kind="ExternalInput")
    wp_d = nc.dram_tensor("wp", [NH * HD, D], F32R, kind="ExternalInput")
    bqk_d = nc.dram_tensor("bqk", [2 * NH * HD], F32, kind="ExternalInput")
    bv_d = nc.dram_tensor("bv", [NH * (HD + 1)], F32, kind="ExternalInput")
    bp_d = nc.dram_tensor("bp", [D], F32, kind="ExternalInput")
    out_d = nc.dram_tensor("out", [S, D], F32, kind="ExternalOutput")

    with TileContext(nc) as tc:
        import contextlib

        stack = contextlib.ExitStack()
        with stack:
            consts = stack.enter_context(tc.tile_pool(name="consts", bufs=1))
            bigs = stack.enter_context(tc.tile_pool(name="bigs", bufs=1))

            # ---- constants (tiles; DMAs are placed to keep x loads first) ----
            wp_sb = consts.tile([P, 2, D], F32R)
            bqk_sb = consts.tile([P, 4], F32)
            bv_sb = consts.tile([1, NH * (HD + 1)], F32)
            bv_rep = consts.tile([P, NH * (HD + 1)], F32)
            bp_sb = consts.tile([1, D], F32)
            bp_rep = consts.tile([P, D], F32)
            ident = consts.tile([P, P], F32)
            make_identity(nc, ident)
            ones_col = consts.tile([1, HD], F32R)
            nc.vector.tensor_scalar(
                out=ones_col,
                in0=ident[0:1, :HD],
                scalar1=0.0,
                scalar2=1.0,
                op0=mybir.AluOpType.mult,
                op1=mybir.AluOpType.add,
            )

            # ---- persistent activations (live across both stages) ----
            qkT_sb = bigs.tile([P, 4, S], F32R)  # q: tiles 0-1, k: tiles 2-3
            v_sb = bigs.tile([P, ST, NH * (HD + 1)], F32R)  # [s%128, s//128, h*65+c]

            v4 = v_sb.rearrange("p st (h c) -> p st h c", c=HD + 1)
            # ones column for the fused row-sum: out = src*0 + 1 (memset can't
            # write f32r; DVE rounds on write). ident is just an initialized src.
            nc.vector.tensor_scalar(
                out=v4[:, :, :, HD : HD + 1],
                in0=ident[:, 0:1, None, None].to_broadcast((P, ST, NH, 1)),
                scalar1=0.0,
                scalar2=1.0,
                op0=mybir.AluOpType.mult,
                op1=mybir.AluOpType.add,
            )

            # ================= stage A: transpose x + projections ==========
            with (
                tc.tile_pool(name="stageA", bufs=1) as pa,
                tc.tile_pool(name="psA", bufs=1, space="PSUM") as psA,
            ):
                wqk_sb = pa.tile([P, DT, 2 * NH * HD], F32R)
                wv_sb = pa.tile([P, DT, NH * HD], F32R)
                xT_sb = pa.tile([P, DT, S], F32R)  # [d%128, d//128, s]
                for sg in range(NCHUNK):
                    for st4 in range(4):
                        st = 4 * sg + st4
                        x_t = pa.tile([P, D], F32, name="x_t", bufs=XT_BUFS)
                        nc.sync.dma_start(x_t[:, :CH], x_d[ts(st, P), :CH])
                        nc.sync.dma_start(x_t[:, CH:], x_d[ts(st, P), CH:])
                        if sg == 0 and st4 == 1:
                            nc.sync.dma_start(
                                wqk_sb, wqk_d[:].rearrange("(do p) f -> p do f", p=P)
                            )
                        if sg == 0 and st4 == 2:
                            nc.sync.dma_start(
                                wv_sb, wv_d[:].rearrange("(do p) f -> p do f", p=P)
                            )
                        if sg == 0 and st4 == 3:
                            nc.sync.dma_start(
                                bqk_sb, bqk_d[:].rearrange("(t p) -> p t", p=P)
                            )
                            nc.sync.dma_start(bv_sb, bv_d[:].unsqueeze(0))
                            nc.gpsimd.partition_broadcast(bv_rep, bv_sb)
                        for bank in range(2):
                            tp = psA.tile([P, CH], F32, name="tp", tag="tp", bufs=TP_BUFS)
                            for q in range(4):
                                do = 4 * bank + q
                                nc.tensor.transpose(
                                    tp[:, ts(q, P)], x_t[:, ts(do, P)], ident
                                )
                            nc.vector.tensor_copy(
                                out=xT_sb[:, 4 * bank : 4 * bank + 4, ts(st, P)],
                                in_=tp.rearrange("p (a b) -> p a b", b=P),
                            )
                    # qk projection for this 512-wide s-chunk
                    for ft in range(4):
                        pp = psA.tile([P, CH], F32, name="pp", tag="proj", bufs=PROJ_BUFS)
                        for do in range(DT):
                            nc.tensor.matmul(
                                pp,
                                (wqk_sb[:, do, ts(ft, P)]),
                                (xT_sb[:, do, ds(sg * CH, CH)]),
                                start=(do == 0),
                                stop=(do == DT - 1),
                            )
                        nc.vector.tensor_scalar_add(
                            qkT_sb[:, ft, ds(sg * CH, CH)], pp, bqk_sb[:, ft : ft + 1]
                        )
                    # v projection for the 4 s-tiles of this chunk
                    for st4 in range(4):
                        st = 4 * sg + st4
                        pv = psA.tile([P, NH * HD], F32, name="pv", tag="projv", bufs=PROJV_BUFS)
                        for do in range(DT):
                            nc.tensor.matmul(
                                pv,
                                (xT_sb[:, do, ts(st, P)]),
                                (wv_sb[:, do, :]),
                                start=(do == 0),
                                stop=(do == DT - 1),
                            )
                        nc.vector.tensor_tensor(
                            v4[:, st, :, :HD],
                            pv.rearrange("p (h c) -> p h c", c=HD),
                            bv_rep.rearrange("p (h c) -> p h c", c=HD + 1)[:, :, :HD],
                            mybir.AluOpType.add,
                        )

            # ================= stage B: attention + output projection ======
            with (
                tc.tile_pool(name="stageB", bufs=1) as pb,
                tc.tile_pool(name="psB", bufs=1, space="PSUM") as psB,
            ):
                nc.sync.dma_start(wp_sb, wp_d[:].rearrange("(i p) f -> p i f", p=P))
                nc.sync.dma_start(bp_sb, bp_d[:].unsqueeze(0))
                nc.gpsimd.partition_broadcast(bp_rep, bp_sb)
                yT_sb = pb.tile([P, 2, S], F32R)  # [dh%128, dh//128, s]
                LP = LP_PAIRS  # pair lookahead
                for j in range(NCHUNK):
                    nsk = 4 * j + 4
                    npair = nsk // 2
                    # band pairs (need masking) first, then the dense pairs
                    pairs = [(4 * j, 4 * j + 1), (4 * j + 2, 4 * j + 3)] + [
                        (i, i + 1) for i in range(0, 4 * j, 2)
                    ]
                    o_ts = {}

                    def emit_outproj(i2):
                        # half-projection over dh tile i2 (heads 2*i2, 2*i2+1)
                        for st4 in range(4):
                            st = 4 * j + st4
                            if i2 == 0:
                                o_ts[st4] = pb.tile([P, D], F32, name="o_t", bufs=OT_BUFS)
                            o_t = o_ts[st4]
                            for c in range(2):
                                op = psB.tile(
                                    [P, CH], F32, name="op", tag="pv", bufs=PV_BUFS
                                )
                                nc.tensor.matmul(
                                    op,
                                    (yT_sb[:, i2, ts(st, P)]),
                                    (wp_sb[:, i2, ds(c * CH, CH)]),
                                    start=True,
                                    stop=True,
                                )
                                if i2 == 0:
                                    nc.vector.tensor_tensor(
                                        o_t[:, ds(c * CH, CH)],
                                        op,
                                        bp_rep[:, ds(c * CH, CH)],
                                        mybir.AluOpType.add,
                                    )
                                else:
                                    nc.vector.tensor_tensor(
                                        o_t[:, ds(c * CH, CH)],
                                        o_t[:, ds(c * CH, CH)],
                                        op,
                                        mybir.AluOpType.add,
                                    )
                            if i2 == 1:
                                nc.sync.dma_start(out_d[ts(st, P), :], o_t)

                    pending_norm = []

                    def flush_norm():
                        # bc matmul + normalize for heads whose recip is ready;
                        # deferred so the PE never waits on the DVE chain
                        while pending_norm:
                            hN, ysbN, recN = pending_norm.pop(0)
                            bc = psB.tile([P, CH], F32, name="bc", tag="pv", bufs=PV_BUFS)
                            nc.tensor.matmul(
                                bc[:HD], ones_col, recN, start=True, stop=True
                            )
                            nc.vector.tensor_tensor(
                                yT_sb[
                                    (hN % 2) * 64 : (hN % 2) * 64 + 64,
                                    hN // 2,
                                    ds(j * CH, CH),
                                ],
                                ysbN[0:HD, :],
                                bc[:HD],
                                mybir.AluOpType.mult,
                            )

                    for h in range(NH):
                        po = (h % 2) * 64
                        qa = qkT_sb[po : po + 64, h // 2, ds(j * CH, CH)]
                        pvp = psB.tile([P, CH], F32, name="pvp", tag="pv", bufs=PV_BUFS)
                        pts = {}

                        def qk_pair(pidx):
                            i0, i1 = pairs[pidx]
                            sc = psB.tile(
                                [P, 2 * CH], F32, name="sc", tag="sc", bufs=SC_BUFS
                            )
                            for half, i in enumerate((i0, i1)):
                                nc.tensor.matmul(
                                    sc[:, ds(half * CH, CH)],
                                    (qkT_sb[po : po + 64, 2 + h // 2, ts(i, P)]),
                                    (qa),
                                    start=True,
                                    stop=True,
                                )
                            pt = pb.tile(
                                [P, 2 * CH], F32R, name="pt", tag="pt", bufs=PT_BUFS
                            )
                            nc.scalar.activation(
                                pt, sc, mybir.ActivationFunctionType.Exp, scale=0.125
                            )
                            for half, i in enumerate((i0, i1)):
                                if i >= 4 * j:
                                    p_rel = i - 4 * j
                                    # keep where (col - row - 128*p_rel) >= 0
                                    nc.gpsimd.affine_select(
                                        out=pt[:, ds(half * CH, CH)],
                                        in_=pt[:, ds(half * CH, CH)],
                                        compare_op=mybir.AluOpType.is_ge,
                                        fill=0.0,
                                        base=-P * p_rel,
                                        pattern=[[1, CH]],
                                        channel_multiplier=-1,
                                    )
                            pts[pidx] = pt

                        for pidx in range(min(LP, npair)):
                            qk_pair(pidx)
                        if pending_norm:
                            flush_norm()
                        if h == 2:
                            emit_outproj(0)
                        for pidx in range(npair):
                            if pidx + LP < npair:
                                qk_pair(pidx + LP)
                            i0, i1 = pairs[pidx]
                            pt = pts.pop(pidx)
                            nc.tensor.matmul(
                                pvp[: HD + 1],
                                (v_sb[:, i0, ds(h * (HD + 1), HD + 1)]),
                                pt[:, :CH],
                                start=(pidx == 0),
                                stop=False,
                            )
                            nc.tensor.matmul(
                                pvp[: HD + 1],
                                (v_sb[:, i1, ds(h * (HD + 1), HD + 1)]),
                                pt[:, CH:],
                                start=False,
                                stop=(pidx == npair - 1),
                            )
                        ysb = pb.tile([HD + 1, CH], F32, name="ysb", bufs=YSB_BUFS)
                        nc.vector.tensor_copy(ysb, pvp[: HD + 1, :])
                        rec = pb.tile([1, CH], F32R, name="rec", bufs=REC_BUFS)
                        with nc.allow_low_precision(reason="recip feeds f32r bcast"):
                            nc.vector.reciprocal(rec, ysb[HD : HD + 1, :])
                        pending_norm.append((h, ysb, rec))
                    flush_norm()
                    emit_outproj(1)

    nc.compile()
    return nc


def make_in_maps(x, W_attn, b_attn, W_proj, b_proj):
    x = np.ascontiguousarray(np.asarray(x, dtype=np.float32))
    W_attn = np.asarray(W_attn, dtype=np.float32)
    b_attn = np.asarray(b_attn, dtype=np.float32)
    W_proj = np.asarray(W_proj, dtype=np.float32)
    b_proj = np.asarray(b_proj, dtype=np.float32)
    GF = NH * HD  # 256 features per group
    in_maps = []
    for c in range(8):
        b, g = divmod(c, 4)
        sl = slice(g * GF, (g + 1) * GF)
        wqk = np.concatenate(
            [W_attn[:, sl], W_attn[:, D + g * GF : D + (g + 1) * GF]], axis=1
        )
        bqk = np.concatenate([b_attn[sl], b_attn[D + g * GF : D + (g + 1) * GF]])
        wv = W_attn[:, 2 * D + g * GF : 2 * D + (g + 1) * GF]
        bv_flat = b_attn[2 * D + g * GF : 2 * D + (g + 1) * GF]
        bv = np.zeros(NH * (HD + 1), dtype=np.float32)
        for h in range(NH):
            bv[h * (HD + 1) : h * (HD + 1) + HD] = bv_flat[h * HD : (h + 1) * HD]
        in_maps.append(
            {
                "x": np.ascontiguousarray(x[b]),
                "wqk": np.ascontiguousarray(wqk),
                "wv": np.ascontiguousarray(wv),
                "wp": np.ascontiguousarray(W_proj[sl, :]),
                "bqk": np.ascontiguousarray(bqk),
                "bv": bv,
                "bp": (b_proj if g == 0 else np.zeros_like(b_proj)).copy(),
            }
        )
    return in_maps


def kernel(x, W_attn, b_attn, W_proj, b_proj):
    global _nc_cache
    from concourse.bass_utils import run_bass_kernel_spmd

    if _nc_cache is None:
        _nc_cache = build_nc()
    nc = _nc_cache
    in_maps = make_in_maps(x, W_attn, b_attn, W_proj, b_proj)
    res = run_bass_kernel_spmd(nc, in_maps, core_ids=list(range(8)))
    out = np.zeros((B, S, D), dtype=np.float32)
    for c in range(8):
        b = c // 4
        out[b] += res.results[c]["out"]
    return out


# revision 34
# speedup vs baseline: 618.4926x; 1.0076x over previous
"""Causal self-attention on 8 NeuronCores.

Sharding: data-parallel over batch (B=2) x tensor-parallel over heads
(16 heads -> 4 groups of 4), Megatron-style. Core c handles batch c//4,
head-group c%4. Each core computes its QKV projection slice, 4 heads of
causal attention, and a partial output projection; the host sums the 4
partials per batch element.

Per-core dataflow (S=2048, D=1024, HD=64, 4 local heads):
  x [S,D] --PE transpose--> xT [D,S]
  qkT = W_qk^T x^T  (fp32r matmuls, heads packed 2-per-128-partitions)
  v   = x W_v       (natural layout, + ones column for row-sum trick)
  scoresT[sk,sq] = kT^T qT ; exp on ACT ; causal mask via affine_select
  yT|sums = [v|1]^T @ probsT  (PV matmul fuses softmax denominator)
  y = yT / sums ; partial_out = y^T W_proj + b_proj (b_proj on group-0 cores)
"""

import sys

sys.path.insert(0, "/opt/trn_rl_repo")

import numpy as np

import concourse.bass as bass
import concourse.bacc as bacc
import concourse.mybir as mybir
from concourse.bass import ts, ds
from concourse.masks import make_identity
from concourse.tile import TileContext

B, S, D, H = 2, 2048, 1024, 16
HD = D // H  # 64
NH = 4  # heads per core
P = 128
DT = D // P  # 8 d-tiles
ST = S // P  # 16 s-tiles
NCHUNK = 4  # sq chunks of 512
CH = 512
F32 = mybir.dt.float32
F32R = mybir.dt.float32r
XT_BUFS = 4
SC_BUFS = 3
PT_BUFS = 14
PV_BUFS = 2
TP_BUFS = 3
PROJ_BUFS = 2
PROJV_BUFS = 2
YSB_BUFS = 3
OT_BUFS = 5
REC_BUFS = 2
LP_PAIRS = 2

_nc_cache = None


def r(ap):
    return ap.bitcast(F32R)


def build_nc():
    nc = bacc.Bacc("TRN2", target_bir_lowering=False, debug=False, num_devices=8)

    x_d = nc.dram_tensor("x", [S, D], F32R, kind="ExternalInput")
    wqk_d = nc.dram_tensor("wqk", [D, 2 * NH * HD], F32R, kind="ExternalInput")
    wv_d = nc.dram_tensor("wv", [D, NH * HD], F32R, kind="ExternalInput")
    wp_d = nc.dram_tensor("wp", [NH * HD, D], F32R, kind="ExternalInput")
    bqk_d = nc.dram_tensor("bqk", [2 * NH * HD], F32, kind="ExternalInput")
    bv_d = nc.dram_tensor("bv", [NH * (HD + 1)], F32, kind="ExternalInput")
    bp_d = nc.dram_tensor("bp", [D], F32, kind="ExternalInput")
    out_d = nc.dram_tensor("out", [S, D], F32, kind="ExternalOutput")

    with TileContext(nc) as tc:
        import contextlib

        stack = contextlib.ExitStack()
        with stack:
            consts = stack.enter_context(tc.tile_pool(name="consts", bufs=1))
            bigs = stack.enter_context(tc.tile_pool(name="bigs", bufs=1))

            # ---- constants (tiles; DMAs are placed to keep x loads first) ----
            wp_sb = consts.tile([P, 2, D], F32R)
            bqk_sb = consts.tile([P, 4], F32)
            bv_sb = consts.tile([1, NH * (HD + 1)], F32)
            bv_rep = consts.tile([P, NH * (HD + 1)], F32)
            bp_sb = consts.tile([1, D], F32)
            bp_rep = consts.tile([P, D], F32)
            ident = consts.tile([P, P], F32)
            make_identity(nc, ident)
            ident_r = consts.tile([P, P], F32R)
            nc.vector.tensor_copy(ident_r, ident)
            ones_col = consts.tile([1, HD], F32R)
            nc.vector.tensor_scalar(
                out=ones_col,
                in0=ident[0:1, :HD],
                scalar1=0.0,
                scalar2=1.0,
                op0=mybir.AluOpType.mult,
                op1=mybir.AluOpType.add,
            )

            # ---- persistent activations (live across both stages) ----
            qkT_sb = bigs.tile([P, 4, S], F32R)  # q: tiles 0-1, k: tiles 2-3
            v_sb = bigs.tile([P, ST, NH * (HD + 1)], F32R)  # [s%128, s//128, h*65+c]

            v4 = v_sb.rearrange("p st (h c) -> p st h c", c=HD + 1)
            # ones column for the fused row-sum: out = src*0 + 1 (memset can't
            # write f32r; DVE rounds on write). ident is just an initialized src.
            nc.vector.tensor_scalar(
                out=v4[:, :, :, HD : HD + 1],
                in0=ident[:, 0:1, None, None].to_broadcast((P, ST, NH, 1)),
                scalar1=0.0,
                scalar2=1.0,
                op0=mybir.AluOpType.mult,
                op1=mybir.AluOpType.add,
            )

            # ================= stage A: transpose x + projections ==========
            with (
                tc.tile_pool(name="stageA", bufs=1) as pa,
                tc.tile_pool(name="psA", bufs=1, space="PSUM") as psA,
            ):
                wqk_sb = pa.tile([P, DT, 2 * NH * HD], F32R)
                wv_sb = pa.tile([P, DT, NH * HD], F32R)
                xT_sb = pa.tile([P, DT, S], F32R)  # [d%128, d//128, s]
                for sg in range(NCHUNK):
                    for st4 in range(4):
                        st = 4 * sg + st4
                        x_t = pa.tile([P, D], F32R, name="x_t", bufs=XT_BUFS)
                        nc.sync.dma_start(x_t[:, :CH], x_d[ts(st, P), :CH])
                        nc.sync.dma_start(x_t[:, CH:], x_d[ts(st, P), CH:])
                        if sg == 0 and st4 == 1:
                            nc.sync.dma_start(
                                wqk_sb, wqk_d[:].rearrange("(do p) f -> p do f", p=P)
                            )
                        if sg == 0 and st4 == 2:
                            nc.sync.dma_start(
                                wv_sb, wv_d[:].rearrange("(do p) f -> p do f", p=P)
                            )
                        if sg == 0 and st4 == 3:
                            nc.sync.dma_start(
                                bqk_sb, bqk_d[:].rearrange("(t p) -> p t", p=P)
                            )
                            nc.sync.dma_start(bv_sb, bv_d[:].unsqueeze(0))
                            nc.gpsimd.partition_broadcast(bv_rep, bv_sb)
                        for bank in range(2):
                            tp = psA.tile([P, CH], F32R, name="tp", tag="tp", bufs=TP_BUFS)
                            for q in range(4):
                                do = 4 * bank + q
                                nc.tensor.transpose(
                                    tp[:, ts(q, P)], x_t[:, ts(do, P)], ident_r
                                )
                            nc.vector.tensor_copy(
                                out=xT_sb[:, 4 * bank : 4 * bank + 4, ts(st, P)],
                                in_=tp.rearrange("p (a b) -> p a b", b=P),
                            )
                    # qk projection for this 512-wide s-chunk
                    for ft in range(4):
                        pp = psA.tile([P, CH], F32, name="pp", tag="proj", bufs=PROJ_BUFS)
                        for do in range(DT):
                            nc.tensor.matmul(
                                pp,
                                (wqk_sb[:, do, ts(ft, P)]),
                                (xT_sb[:, do, ds(sg * CH, CH)]),
                                start=(do == 0),
                                stop=(do == DT - 1),
                            )
                        nc.vector.tensor_scalar_add(
                            qkT_sb[:, ft, ds(sg * CH, CH)], pp, bqk_sb[:, ft : ft + 1]
                        )
                    # v projection for the 4 s-tiles of this chunk
                    for st4 in range(4):
                        st = 4 * sg + st4
                        pv = psA.tile([P, NH * HD], F32, name="pv", tag="projv", bufs=PROJV_BUFS)
                        for do in range(DT):
                            nc.tensor.matmul(
                                pv,
                                (xT_sb[:, do, ts(st, P)]),
                                (wv_sb[:, do, :]),
                                start=(do == 0),
                                stop=(do == DT - 1),
                            )
                        nc.vector.tensor_tensor(
                            v4[:, st, :, :HD],
                            pv.rearrange("p (h c) -> p h c", c=HD),
                            bv_rep.rearrange("p (h c) -> p h c", c=HD + 1)[:, :, :HD],
                            mybir.AluOpType.add,
                        )

            # ================= stage B: attention + output projection ======
            with (
                tc.tile_pool(name="stageB", bufs=1) as pb,
                tc.tile_pool(name="psB", bufs=1, space="PSUM") as psB,
            ):
                nc.sync.dma_start(wp_sb, wp_d[:].rearrange("(i p) f -> p i f", p=P))
                nc.sync.dma_start(bp_sb, bp_d[:].unsqueeze(0))
                nc.gpsimd.partition_broadcast(bp_rep, bp_sb)
                yT_sb = pb.tile([P, 2, S], F32R)  # [dh%128, dh//128, s]
                LP = LP_PAIRS  # pair lookahead
                for j in range(NCHUNK):
                    nsk = 4 * j + 4
                    npair = nsk // 2
                    # band pairs (need masking) first, then the dense pairs
                    pairs = [(4 * j, 4 * j + 1), (4 * j + 2, 4 * j + 3)] + [
                        (i, i + 1) for i in range(0, 4 * j, 2)
                    ]
                    o_ts = {}

                    def emit_outproj(i2):
                        # half-projection over dh tile i2 (heads 2*i2, 2*i2+1)
                        for st4 in range(4):
                            st = 4 * j + st4
                            if i2 == 0:
                                o_ts[st4] = pb.tile([P, D], F32, name="o_t", bufs=OT_BUFS)
                            o_t = o_ts[st4]
                            for c in range(2):
                                op = psB.tile(
                                    [P, CH], F32, name="op", tag="pv", bufs=PV_BUFS
                                )
                                nc.tensor.matmul(
                                    op,
                                    (yT_sb[:, i2, ts(st, P)]),
                                    (wp_sb[:, i2, ds(c * CH, CH)]),
                                    start=True,
                                    stop=True,
                                )
                                if i2 == 0:
                                    nc.vector.tensor_tensor(
                                        o_t[:, ds(c * CH, CH)],
                                        op,
                                        bp_rep[:, ds(c * CH, CH)],
                                        mybir.AluOpType.add,
                                    )
                                else:
                                    nc.vector.tensor_tensor(
                                        o_t[:, ds(c * CH, CH)],
                                        o_t[:, ds(c * CH, CH)],
                                        op,
                                        mybir.AluOpType.add,
                                    )
                            if i2 == 1:
                                nc.sync.dma_start(out_d[ts(st, P), :], o_t)

                    pending_norm = []

                    def flush_norm():
                        # bc matmul + normalize for heads whose recip is ready;
                        # deferred so the PE never waits on the DVE chain
                        while pending_norm:
                            hN, ysbN, recN = pending_norm.pop(0)
                            bc = psB.tile([P, CH], F32, name="bc", tag="pv", bufs=PV_BUFS)
                            nc.tensor.matmul(
                                bc[:HD], ones_col, recN, start=True, stop=True
                            )
                            nc.vector.tensor_tensor(
                                yT_sb[
                                    (hN % 2) * 64 : (hN % 2) * 64 + 64,
                                    hN // 2,
                                    ds(j * CH, CH),
                                ],
                                ysbN[0:HD, :],
                                bc[:HD],
                                mybir.AluOpType.mult,
                            )

                    for h in range(NH):
                        po = (h % 2) * 64
                        qa = qkT_sb[po : po + 64, h // 2, ds(j * CH, CH)]
                        pvp = psB.tile([P, CH], F32, name="pvp", tag="pv", bufs=PV_BUFS)
                        pts = {}

                        def qk_pair(pidx):
                            i0, i1 = pairs[pidx]
                            sc = psB.tile(
                                [P, 2 * CH], F32, name="sc", tag="sc", bufs=SC_BUFS
                            )
                            for half, i in enumerate((i0, i1)):
                                nc.tensor.matmul(
                                    sc[:, ds(half * CH, CH)],
                                    (qkT_sb[po : po + 64, 2 + h // 2, ts(i, P)]),
                                    (qa),
                                    start=True,
                                    stop=True,
                                )
                            pt = pb.tile(
                                [P, 2 * CH], F32R, name="pt", tag="pt", bufs=PT_BUFS
                            )
                            nc.scalar.activation(
                                pt, sc, mybir.ActivationFunctionType.Exp, scale=0.125
                            )
                            for half, i in enumerate((i0, i1)):
                                if i >= 4 * j:
                                    p_rel = i - 4 * j
                                    # keep where (col - row - 128*p_rel) >= 0
                                    nc.gpsimd.affine_select(
                                        out=pt[:, ds(half * CH, CH)],
                                        in_=pt[:, ds(half * CH, CH)],
                                        compare_op=mybir.AluOpType.is_ge,
                                        fill=0.0,
                                        base=-P * p_rel,
                                        pattern=[[1, CH]],
                                        channel_multiplier=-1,
                                    )
                            pts[pidx] = pt

                        for pidx in range(min(LP, npair)):
                            qk_pair(pidx)
                        if pending_norm:
                            flush_norm()
                        if h == 2:
                            emit_outproj(0)
                        for pidx in range(npair):
                            if pidx + LP < npair:
                                qk_pair(pidx + LP)
                            i0, i1 = pairs[pidx]
                            pt = pts.pop(pidx)
                            nc.tensor.matmul(
                                pvp[: HD + 1],
                                (v_sb[:, i0, ds(h * (HD + 1), HD + 1)]),
                                pt[:, :CH],
                                start=(pidx == 0),
                                stop=False,
                            )
                            nc.tensor.matmul(
                                pvp[: HD + 1],
                                (v_sb[:, i1, ds(h * (HD + 1), HD + 1)]),
                                pt[:, CH:],
                                start=False,
                                stop=(pidx == npair - 1),
                            )
                        ysb = pb.tile([HD + 1, CH], F32, name="ysb", bufs=YSB_BUFS)
                        nc.vector.tensor_copy(ysb, pvp[: HD + 1, :])
                        rec = pb.tile([1, CH], F32R, name="rec", bufs=REC_BUFS)
                        with nc.allow_low_precision(reason="recip feeds f32r bcast"):
                            nc.vector.reciprocal(rec, ysb[HD : HD + 1, :])
                        pending_norm.append((h, ysb, rec))
                    flush_norm()
                    emit_outproj(1)

    nc.compile()
    return nc


def make_in_maps(x, W_attn, b_attn, W_proj, b_proj):
    x = np.ascontiguousarray(np.asarray(x, dtype=np.float32))
    W_attn = np.asarray(W_attn, dtype=np.float32)
    b_attn = np.asarray(b_attn, dtype=np.float32)
    W_proj = np.asarray(W_proj, dtype=np.float32)
    b_proj = np.asarray(b_proj, dtype=np.float32)
    GF = NH * HD  # 256 features per group
    in_maps = []
    for c in range(8):
        b, g = divmod(c, 4)
        sl = slice(g * GF, (g + 1) * GF)
        wqk = np.concatenate(
            [W_attn[:, sl], W_attn[:, D + g * GF : D + (g + 1) * GF]], axis=1
        )
        bqk = np.concatenate([b_attn[sl], b_attn[D + g * GF : D + (g + 1) * GF]])
        wv = W_attn[:, 2 * D + g * GF : 2 * D + (g + 1) * GF]
        bv_flat = b_attn[2 * D + g * GF : 2 * D + (g + 1) * GF]
        bv = np.zeros(NH * (HD + 1), dtype=np.float32)
        for h in range(NH):
            bv[h * (HD + 1) : h * (HD + 1) + HD] = bv_flat[h * HD : (h + 1) * HD]
        in_maps.append(
            {
                "x": np.ascontiguousarray(x[b]),
                "wqk": np.ascontiguousarray(wqk),
                "wv": np.ascontiguousarray(wv),
                "wp": np.ascontiguousarray(W_proj[sl, :]),
                "bqk": np.ascontiguousarray(bqk),
                "bv": bv,
                "bp": (b_proj if g == 0 else np.zeros_like(b_proj)).copy(),
            }
        )
    return in_maps


def kernel(x, W_attn, b_attn, W_proj, b_proj):
    global _nc_cache
    from concourse.bass_utils import run_bass_kernel_spmd

    if _nc_cache is None:
        _nc_cache = build_nc()
    nc = _nc_cache
    in_maps = make_in_maps(x, W_attn, b_attn, W_proj, b_proj)
    res = run_bass_kernel_spmd(nc, in_maps, core_ids=list(range(8)))
    out = np.zeros((B, S, D), dtype=np.float32)
    for c in range(8):
        b = c // 4
        out[b] += res.results[c]["out"]
    return out
